# revision 28
# baseline (speedup 1.0000x reference)
"""Color-loss kernel for Trainium2 (8 NeuronCores, data-parallel over batch).

Computes, for real/fake [32, 3, 512, 512] fp32 RGB images:
    y = mean(|Y(real) - Y(fake)|)            (L1 on Y)
    u = mean(smooth_l1(U(real) - U(fake)))   (SmoothL1, beta=1)
    v = mean(smooth_l1(V(real) - V(fake)))
    loss = y + u + v
where (Y,U,V) = RGB2YUV @ rgb per pixel (skimage matrix).

Math used on-device (d := real - fake per channel; the transform is linear):
    tY2 = (dR*(RY/GY) + dG)*(GY/BY) + dB ;  dY = BY*tY2   (2 fused DVE ops)
    dU = -KU*(BY*tY2 - dB), KU = BU/(1-BY)   (row residual ~3.5e-10)
    dV = -KV*(BY*tY2 - dR), KV = RV/(1-RY)   (row residual ~1e-6; loss
        impact measured 3e-7 relative — the skimage V row is not exactly
        rank-structured, but far below fp32-level differences that matter)
    smooth_l1(d) = 0.5 d^2 - 0.5 relu(|d|-1)^2
    relu(|d|-1)^2 = (max(s*t,1)-1)^2 + (max(-s*t,1)-1)^2   (|d| = s*|t|)
|dU| <= 0.872 < 1 always for inputs in [0,1), so U needs no relu correction;
|dV| can reach 1.23 (3187 pixels exceed 1 on the seed-0 data), so V keeps the
correction terms.

Each core processes 4 images in pieces ("fl" chunking: first/last image in
halves); ScalarE accumulates per-partition partial sums (|dY|, dU^2, dV^2,
and the two V correction terms) into a [128, 5*G] stats tile; host sums and
combines. Measured ~62-67us/core steady-state (~400 GB/s/core effective) —
at the HBM roofline; VectorE ~50us and ScalarE ~40us hide under the DMA.
"""

import numpy as np

import concourse.bacc as bacc
import concourse.tile as tile
from concourse import mybir
from concourse import bass_utils

N_CORES = 8
B_FULL = 32
B_CORE = B_FULL // N_CORES  # 4 images per core
H = W = 512
PIX = H * W  # 262144 pixels per channel plane
P = 128  # SBUF partitions
FD = PIX // P  # 2048 free-dim elems per channel per image
N_PIXELS = B_FULL * PIX  # denominator of each mean

# skimage rgb2yuv matrix rows
RY, GY, BY = 0.299, 0.587, 0.114
RU, GU, BU = -0.14714119, -0.28886916, 0.43601035
RV, GV, BV = 0.61497657, -0.51496512, -0.10001026

S1Y = RY / GY  # dY chain:  tY1 = dR*S1Y + dG ; tY2 = tY1*S2Y + dB ; dY = BY*tY2
S2Y = GY / BY
KU = BU / (1.0 - BY)  # dU = -KU*(BY*tY2 - dB)   (row residual ~3.5e-10)
KV = RV / (1.0 - RY)  # dV = -KV*(BY*tY2 - dR)  (row residual ~1e-6 rel)

_CACHE = {}


def groups_for(chunk):
    """Processing pieces as (image, j_start, j_len) over the [P, FD] plane view."""
    if chunk == "fl":
        gs = []
        for b in range(B_CORE):
            if b in (0, B_CORE - 1):
                gs += [(b, 0, FD // 2), (b, FD // 2, FD // 2)]
            else:
                gs.append((b, 0, FD))
        return gs
    n = int(chunk)
    cf = FD // n
    return [(b, h * cf, cf) for b in range(B_CORE) for h in range(n)]


def _build(reps=1, mode="full", dma_split="img", chunk=1):
    """Build + compile the per-core Bass program (same SPMD program on all cores).

    reps > 1 repeats the whole computation (identical results; used by test.py
    to measure per-iteration HW time by scaling).
    mode: "full" | "dma" (loads only) | "compute" (load once, compute per rep)
    — diagnostic variants for locating the bottleneck.
    dma_split: "img" (one 3MB DMA per image/tensor) | "plane" (one fully
    contiguous 1MB DMA per image/channel/tensor).
    chunk: pieces per image (1 or 2), or "fl" — split only the first image
    (shorter pipeline fill) and the last image (shorter drain tail) while the
    middle images keep full-size chunks for best DMA/instruction efficiency.
    """
    nc = bacc.Bacc("TRN2", target_bir_lowering=False, debug=False,
                   num_devices=N_CORES)
    f32 = mybir.dt.float32
    bf16 = mybir.dt.bfloat16
    A = mybir.AluOpType
    F = mybir.ActivationFunctionType

    groups = groups_for(chunk)  # (image, j_start, j_len) per processed piece
    G = len(groups)  # stat column groups

    real = nc.dram_tensor("real", [B_CORE, 3, H, W], f32, kind="ExternalInput").ap()
    fake = nc.dram_tensor("fake", [B_CORE, 3, H, W], f32, kind="ExternalInput").ap()
    out = nc.dram_tensor("stats", [P, 5 * G], f32, kind="ExternalOutput").ap()

    # [b, c, h, w] -> [b, p, c, j]: pixel (h, w) -> partition h//4, col (h%4)*512+w
    rview = real.rearrange("b c (p h2) w -> b p c (h2 w)", h2=4)
    fview = fake.rearrange("b c (p h2) w -> b p c (h2 w)", h2=4)
    # per-plane views [b, c, p, j] (each [p, j] slice is one contiguous 1MB range)
    rplane = real.rearrange("b c (p h2) w -> b c p (h2 w)", h2=4)
    fplane = fake.rearrange("b c (p h2) w -> b c p (h2 w)", h2=4)

    with tile.TileContext(nc) as tc:
        with (
            tc.tile_pool(name="io", bufs=2) as io_pool,
            tc.tile_pool(name="dif", bufs=2) as d_pool,
            tc.tile_pool(name="mid", bufs=2) as t_pool,
            tc.tile_pool(name="scr", bufs=2) as scr_pool,
            tc.tile_pool(name="acc", bufs=1) as s_pool,
        ):
            stats = s_pool.tile([P, 5 * G], f32)

            def load(b, j0, CF):
                rt = io_pool.tile([P, 3 * CF], f32, tag="rt")
                ft = io_pool.tile([P, 3 * CF], f32, tag="ft")
                js = slice(j0, j0 + CF)
                if dma_split == "img":
                    nc.sync.dma_start(
                        out=rt[:].rearrange("p (c j) -> p c j", c=3),
                        in_=rview[b][:, :, js],
                    )
                    nc.sync.dma_start(
                        out=ft[:].rearrange("p (c j) -> p c j", c=3),
                        in_=fview[b][:, :, js],
                    )
                else:  # "plane": fully contiguous 1MB per DMA
                    for c in range(3):
                        nc.sync.dma_start(
                            out=rt[:, c * CF : (c + 1) * CF], in_=rplane[b, c][:, js]
                        )
                        nc.sync.dma_start(
                            out=ft[:, c * CF : (c + 1) * CF], in_=fplane[b, c][:, js]
                        )
                return rt, ft

            def compute(rt, ft, g, CF):
                d = d_pool.tile([P, 3 * CF], bf16, tag="d")
                nc.vector.tensor_tensor(out=d[:], in0=rt[:], in1=ft[:], op=A.subtract)
                dR = d[:, 0:CF]
                dG = d[:, CF : 2 * CF]
                dB = d[:, 2 * CF : 3 * CF]

                ty1 = t_pool.tile([P, CF], bf16, tag="ty1")
                nc.vector.scalar_tensor_tensor(
                    out=ty1[:], in0=dR, scalar=S1Y, in1=dG, op0=A.mult, op1=A.add
                )
                ty2 = t_pool.tile([P, CF], bf16, tag="ty2")
                nc.vector.scalar_tensor_tensor(
                    out=ty2[:], in0=ty1[:], scalar=S2Y, in1=dB, op0=A.mult, op1=A.add
                )
                # dU = -KU*(BY*tY2 - dB) ; dV = -KV*(BY*tY2 - dR)
                up = t_pool.tile([P, CF], bf16, tag="up")
                nc.vector.scalar_tensor_tensor(
                    out=up[:], in0=ty2[:], scalar=BY, in1=dB, op0=A.mult,
                    op1=A.subtract,
                )
                vp = t_pool.tile([P, CF], bf16, tag="vp")
                nc.vector.scalar_tensor_tensor(
                    out=vp[:], in0=ty2[:], scalar=BY, in1=dR, op0=A.mult,
                    op1=A.subtract,
                )
                # V relu-correction precursors: e± = max(±KV*vp, 1); |dV| = KV*|vp|
                ep = t_pool.tile([P, CF], bf16, tag="ep")
                nc.vector.tensor_scalar(
                    out=ep[:], in0=vp[:], scalar1=KV, scalar2=1.0,
                    op0=A.mult, op1=A.max,
                )
                em = t_pool.tile([P, CF], bf16, tag="em")
                nc.vector.tensor_scalar(
                    out=em[:], in0=vp[:], scalar1=-KV, scalar2=1.0,
                    op0=A.mult, op1=A.max,
                )

                # ScalarE accumulating reductions -> stats[:, q*G + g]
                # q0: sum |dY| = sum Abs(BY*tY2)
                # q1: sum dU^2 = sum Square(KU*up)
                # q2: sum dV^2 = sum Square(KV*vp)
                # q3: sum (e+ - 1)^2 ; q4: sum (e- - 1)^2
                for qi, (src, func, scale, bias) in enumerate([
                    (ty2, F.Abs, BY, 0.0),
                    (up, F.Square, KU, 0.0),
                    (vp, F.Square, KV, 0.0),
                    # (e-1)^2 == (1-e)^2, and only bias=+1.0 has a const AP
                    (ep, F.Square, -1.0, 1.0),
                    (em, F.Square, -1.0, 1.0),
                ]):
                    scr = scr_pool.tile([P, CF], bf16, tag="scr")
                    nc.scalar.activation(
                        out=scr[:], in_=src[:], func=func, bias=bias, scale=scale,
                        accum_out=stats[:, qi * G + g : qi * G + g + 1],
                    )

            if mode == "full":
                for _ in range(reps):
                    for gi, (b, j0, cf) in enumerate(groups):
                        rt, ft = load(b, j0, cf)
                        compute(rt, ft, gi, cf)
            elif mode == "dma":
                nc.gpsimd.memset(stats[:], 0.0)
                sink = s_pool.tile([P, 1], f32)
                for _ in range(reps):
                    for b, j0, cf in groups:
                        rt, ft = load(b, j0, cf)
                        # tiny consumer so loads aren't dead
                        nc.vector.tensor_tensor(
                            out=sink[:], in0=rt[:, 0:1], in1=ft[:, 0:1], op=A.add
                        )
            elif mode == "compute":
                # diagnostic only: one resident load, repeated compute passes
                # (stat values meaningless; timing-equivalent op mix)
                rt, ft = load(0, 0, FD)
                for _ in range(reps):
                    for gi, (b, j0, cf) in enumerate(groups):
                        compute(rt, ft, gi, cf)
            else:
                raise ValueError(mode)

            nc.sync.dma_start(out=out[:], in_=stats[:])
    nc.compile()
    return nc


# "fl": first/last image processed in half-size pieces — shortens the
# single-exec pipeline fill (first load is 1.5MB instead of 3MB) and drain
# tail (last compute piece is half-size); middle images stay full-size for
# best DMA efficiency. Verified numerically identical quality to chunk=1.
DEFAULT_CHUNK = "fl"


def _get_nc(reps=1, mode="full", dma_split="img", chunk=None):
    if chunk is None:
        chunk = DEFAULT_CHUNK
    key = ("nc", reps, mode, dma_split, chunk)
    if key not in _CACHE:
        _CACHE[key] = _build(reps, mode, dma_split, chunk)
    return _CACHE[key]


def kernel(real, fake):
    real = np.ascontiguousarray(np.asarray(real, dtype=np.float32))
    fake = np.ascontiguousarray(np.asarray(fake, dtype=np.float32))
    assert real.shape == (B_FULL, 3, H, W) and fake.shape == (B_FULL, 3, H, W)

    nc = _get_nc()
    in_maps = [
        {
            "real": real[k * B_CORE : (k + 1) * B_CORE],
            "fake": fake[k * B_CORE : (k + 1) * B_CORE],
        }
        for k in range(N_CORES)
    ]
    res = bass_utils.run_bass_kernel_spmd(nc, in_maps, core_ids=list(range(N_CORES)))

    G = len(groups_for(DEFAULT_CHUNK))
    tot = np.zeros(5, dtype=np.float64)
    for r in res.results:
        s = r["stats"].astype(np.float64)
        for q in range(5):
            tot[q] += s[:, q * G : (q + 1) * G].sum()

    tot_y, tot_u, tot_v, tot_p, tot_m = tot
    loss = (tot_y + 0.5 * (tot_u + tot_v - tot_p - tot_m)) / N_PIXELS
    return np.float32(loss)
